# revision 24
# baseline (speedup 1.0000x reference)
"""Trainium2 Bass kernel for CategoricalDnn: embedding gather + BatchNorm(train) + ReLU + concat.

Reference computation (B=65536, F=32, V=1001, D=64, N_NUM=16):
    emb[b,f,:]  = tables[f, cat_idx[b,f], :]
    mean/var    = biased batch stats of emb over b
    normed      = (emb - mean) * rsqrt(var+eps) * gamma + beta
    out         = concat([relu(normed).reshape(B, F*D), numerical], axis=1)

Strategy (8 NeuronCores, data-parallel over the batch):
  * Host: linearize cat_idx into int16 row indices of the flattened
    [F*V, 64] table in the wrapped-by-16 layout dma_gather expects, and
    histogram the indices per feature (np.bincount - integer index
    preprocessing only; no table values touched on the host).
  * Single gather pass (device): per 128-row batch tile, four 1024-index
    dma_gather instructions (mlp gpsimd library) pull the 4096 embedding
    rows ([128, 32, 64] = 1 MB) from HBM, one per SWDGE queue -
    descriptor generation on the gpsimd Q7 cluster is the bottleneck
    resource and parallelizes ~3x across the 4 queues. The gpsimd queue
    does nothing else, so it never stalls except on buffer backpressure.
  * EXACT global BN statistics with no stats gather pass, via the
    algebraic identity sum_b emb[b,f,:] = sum_v count[f,v]*table[f,v,:]
    (and with table^2 for the second moment): every core redundantly
    computes all 32 features' moments from a zero-padded [s,g,d]-tiled
    copy of the table and the d-replicated count matrix - 8 iterations
    of DVE products + in-place ACT square + PE column-sum + DVE folds,
    ~17 MB of sequential loads, all overlapped with the first gather
    tiles. Global counts make every core's stats identical, so no
    collective is needed at all.
  * Every tile: x*scale+shift on DVE (scale = gamma*rsqrt(var+eps),
    shift = beta - mean*scale, PE-broadcast to 128 partitions), ReLU on
    ACT, splice the numerical columns, store [128, 2064] blocks on HWDGE.
"""

import sys

import numpy as np

if "/opt/trn_rl_repo" not in sys.path:
    sys.path.insert(0, "/opt/trn_rl_repo")

import concourse.bacc as bacc
import concourse.bass as bass
import concourse.mybir as mybir
from concourse.bass_utils import run_bass_kernel_spmd
from concourse.library_config import mlp as mlp_library

# Problem constants (hardcoded per harness contract).
B, F, V, D, N_NUM = 65536, 32, 1001, 64, 16
EPS = 1e-5
NCORES = 8
BC = B // NCORES          # 8192 batch rows per core
TILE = 128                # batch rows per gather tile
NT = BC // TILE           # 64 tiles per core
FD = F * D                # 2048
OW = FD + N_NUM           # 2064 output columns
R = F * V                 # 32032 flat table rows (< int16 max)
P = 128
NIDX = TILE * F           # 4096 gathered rows per tile
IW = NIDX // 16           # 256 idx columns per tile (wrapped by 16)
NSUB = 4                  # sub-gathers per tile (1024 idxs each: packed max)
SIDX = NIDX // NSUB       # 1024 gathered rows per sub-gather
SIW = IW // NSUB          # 64 idx columns per sub-gather
SF = F // NSUB            # 8 features per sub-gather

NB2 = 8                   # gather/store buffers
NQ = 4                    # SWDGE queues for gather desc-gen parallelism
VP = 1024                 # padded vocab for the stats tiling
NG = 8                    # stat groups (4 features each)
GF = F // NG              # 4 features per stat group
GW = GF * 8 * D           # 2048 stat columns per group ([s, g, d] tiled)

f32 = mybir.dt.float32
i16 = mybir.dt.int16


def _build_nc() -> bass.Bass:
    nc = bacc.Bacc("TRN2", target_bir_lowering=False, debug=False,
                   num_devices=NCORES, num_swdge_queues=NQ)

    tab = nc.dram_tensor("tab", [R, D], f32, kind="ExternalInput")
    idx = nc.dram_tensor("idx", [P, NT * IW], i16, kind="ExternalInput")
    nume = nc.dram_tensor("nume", [P, NT * N_NUM], f32, kind="ExternalInput")
    gb = nc.dram_tensor("gb", [P, 2 * FD], f32, kind="ExternalInput")
    tabs = nc.dram_tensor("tabs", [P, NG * GW], f32, kind="ExternalInput")
    cntr = nc.dram_tensor("cntr", [P, NG * GW], f32, kind="ExternalInput")
    out = nc.dram_tensor("out", [BC, OW], f32, kind="ExternalOutput")

    from contextlib import ExitStack
    with ExitStack() as ctx:
        sb = lambda name, shape, dt: ctx.enter_context(
            nc.sbuf_tensor(name, shape, dt))
        idx_sb = sb("idx_sb", [P, NT * IW], i16)
        g2 = [sb(f"g2_{k}", [P, OW], f32) for k in range(NB2)]
        tabg = [sb(f"tabg{k}", [P, GW], f32) for k in range(2)]
        cntg = [sb(f"cntg{k}", [P, GW], f32) for k in range(2)]
        wg = sb("wg", [P, GW], f32)
        wqb = sb("wqb", [P, GW], f32)
        sc_bc = sb("sc_bc", [P, FD], f32)
        sh_bc = sb("sh_bc", [P, FD], f32)
        num_sb = sb("num_sb", [P, NT * N_NUM], f32)
        ga_sb = sb("ga_sb", [P, FD], f32)
        be_sb = sb("be_sb", [P, FD], f32)
        stat_s = sb("stat_s", [1, FD], f32)
        stat_q = sb("stat_q", [1, FD], f32)
        ones_c = sb("ones_c", [P, 1], f32)
        ones_r = sb("ones_r", [1, P], f32)
        eps_col = sb("eps_col", [P, 1], f32)

        ps_q = ctx.enter_context(nc.psum_tensor("ps_q", [1, FD], f32))
        ps_bc = ctx.enter_context(nc.psum_tensor("ps_bc", [P, FD], f32))

        sem = lambda name: ctx.enter_context(nc.semaphore(name))
        s_ld = sem("s_ld")
        s_tg = sem("s_tg")
        s_sq = sem("s_sq")
        s_pe = sem("s_pe")
        s_ax = sem("s_ax")
        s_g2 = [sem(f"s_g2_{k}") for k in range(NB2)]
        s_v2 = sem("s_v2")
        s_r2 = sem("s_r2")
        s_w2 = [sem(f"s_w2_{k}") for k in range(NB2)]
        s_vi = sem("s_vi")
        s_vm = sem("s_vm")
        s_vs = sem("s_vs")

        # tiles mapped to buffer k (NT need not divide NB2)
        cnt2 = lambda k: (NT - k + NB2 - 1) // NB2
        g3q = lambda buf, q: buf[:, q * SF * D:(q + 1) * SF * D].rearrange(
            "p (f d) -> p f d", d=D)

        # vstep indices of the stats pipeline (9 per group, then the math)
        W_J = lambda j: 9 * j            # w product of group j
        CF_J = lambda j: 9 * j + 6       # cnt factor consumed (slot free)
        N_LOOP = 9 * NG                  # 72
        N_VAR = N_LOOP + 8               # var ready after this many steps
        N_VS = N_LOOP + 12               # wide scale/shift ready

        with nc.Block("main") as block:

            @block.sync
            def _(sync):
                sync.dma_start(idx_sb[:, :], idx[:, :]).then_inc(s_ld, 16)
                sync.dma_start(num_sb[:, :], nume[:, :]).then_inc(s_ld, 16)
                sync.dma_start(ga_sb[:, :], gb[:, :FD]).then_inc(s_ld, 16)
                sync.dma_start(be_sb[:, :], gb[:, FD:]).then_inc(s_ld, 16)
                # output stores
                for t in range(NT):
                    sync.wait_ge(s_r2, t + 1)
                    sync.dma_start(
                        out[t * TILE:(t + 1) * TILE, :], g2[t % NB2][:, :],
                    ).then_inc(s_w2[t % NB2], 16)
                for k in range(NB2):
                    sync.wait_ge(s_w2[k], 16 * cnt2(k))

            @block.gpsimd
            def _(gpsimd):
                # pure gather stream: the only stalls are buffer backpressure
                gpsimd.load_library(mlp_library)
                gpsimd.wait_ge(s_ld, 16)  # idx loaded (HWDGE FIFO per engine)
                for t in range(NT):
                    if t >= NB2:
                        gpsimd.wait_ge(s_w2[t % NB2], 16 * (t // NB2))
                    for q in range(NSUB):
                        gpsimd.dma_gather(
                            g3q(g2[t % NB2], q), tab[:, :],
                            idx_sb[:, t * IW + q * SIW:t * IW + (q + 1) * SIW],
                            SIDX, SIDX, D, single_packet=True, queue_num=q,
                        ).then_inc(s_g2[t % NB2], 16)

            @block.vector
            def _(vector):
                # Same-engine RAW/WAW chains need explicit sems (deep DVE
                # pipeline): s_vi orders the boot memsets, s_vs chains the
                # stats section, s_vm/s_v2 the per-tile normalize groups.
                vector.memset(ones_c[:, :], 1.0)
                vector.memset(ones_r[:, :], 1.0)
                vector.memset(eps_col[:, :], float(EPS)).then_inc(s_vi, 1)
                # ---- stats section: every DVE op self-chained via s_vs ----
                vsn = [0]

                def vstep(emit, *waits):
                    vector.wait_ge(s_vs, vsn[0])
                    for w_sem, w_val in waits:
                        vector.wait_ge(w_sem, w_val)
                    inst = emit()
                    vsn[0] += 1
                    inst.then_inc(s_vs, 1)
                    return inst

                def folds(b):
                    # [s(8), g(4), d] tiled columns: halve over s three
                    # times with full-width 128-partition adds, landing the
                    # per-partition (g, d) partials in b[:, :256]
                    vstep(lambda: vector.tensor_add(
                        b[:, :1024], b[:, :1024], b[:, 1024:]))
                    vstep(lambda: vector.tensor_add(
                        b[:, :512], b[:, :512], b[:, 512:1024]))
                    vstep(lambda: vector.tensor_add(
                        b[:, :256], b[:, :256], b[:, 256:512]))

                for j in range(NG):
                    sl = j % 2
                    # w = tab * cnt   (raw first-moment contributions);
                    # waits for mmA of group j-1 to have drained wg
                    vstep(lambda sl=sl: vector.tensor_mul(
                        wg[:, :], tabg[sl][:, :], cntg[sl][:, :]),
                        (s_vi, 1), (s_tg, 32 * (j + 1)),
                        (s_pe, max(0, 2 * j - 1)))
                    folds(wg)         # PE colsum into ps_bc[0:1] follows
                    # wq = tab^2 * cnt in its own buffer (only waits mmB j-1)
                    vstep(lambda sl=sl: vector.tensor_mul(
                        wqb[:, :], tabg[sl][:, :], tabg[sl][:, :]),
                        (s_pe, 2 * j))
                    vstep(lambda sl=sl: vector.tensor_mul(
                        wqb[:, :], wqb[:, :], cntg[sl][:, :]))
                    folds(wqb)        # PE colsum into ps_q follows
                # ---- global stats -> wide scale/shift ----
                # PSUM is only ever read with tensor_copy; scaling happens
                # in SBUF afterwards (tensor_scalar straight from PSUM
                # produced garbage on HW).
                vstep(lambda: vector.tensor_copy(stat_s[:, :], ps_bc[0:1, :]),
                      (s_pe, 2 * NG))                         # 72
                vstep(lambda: vector.tensor_copy(stat_q[:, :], ps_q[:, :]))
                vstep(lambda: vector.tensor_copy(sc_bc[:, :], ps_bc[:, :]),
                      (s_pe, 2 * NG + 1))                     # 74: mean_bc*B
                vstep(lambda: vector.tensor_scalar_mul(
                    sc_bc[:, :], sc_bc[:, :], 1.0 / B))       # 75
                vstep(lambda: vector.tensor_copy(sh_bc[:, :], ps_bc[:, :]),
                      (s_pe, 2 * NG + 2))                     # 76: E[x^2]*B
                vstep(lambda: vector.tensor_scalar_mul(
                    sh_bc[:, :], sh_bc[:, :], 1.0 / B))       # 77
                vstep(lambda: vector.tensor_mul(
                    wg[:, :], sc_bc[:, :], sc_bc[:, :]))      # mean^2
                vstep(lambda: vector.tensor_sub(
                    sh_bc[:, :], sh_bc[:, :], wg[:, :]))      # var (wide)
                vstep(lambda: vector.reciprocal(
                    sh_bc[:, :], sh_bc[:, :]), (s_ax, 1))     # rsqrt
                vstep(lambda: vector.tensor_mul(
                    wg[:, :], ga_sb[:, :], sh_bc[:, :]),
                    (s_ld, 64))                               # SCALE -> wg
                vstep(lambda: vector.tensor_mul(
                    sc_bc[:, :], sc_bc[:, :], wg[:, :]))      # mean*scale
                vstep(lambda: vector.tensor_sub(
                    sc_bc[:, :], be_sb[:, :], sc_bc[:, :]))   # SHIFT -> sc_bc
                assert vsn[0] == N_VS, vsn[0]
                # normalize every tile
                for t in range(NT):
                    if t == 0:
                        vector.wait_ge(s_vs, N_VS)
                    else:
                        vector.wait_ge(s_v2, t)
                    vector.wait_ge(s_g2[t % NB2], 16 * NSUB * (t // NB2 + 1))
                    gt = g2[t % NB2]
                    vector.tensor_mul(
                        gt[:, :FD], gt[:, :FD], wg[:, :]).then_inc(s_vm, 1)
                    vector.wait_ge(s_vm, t + 1)
                    vector.tensor_add(
                        gt[:, :FD], gt[:, :FD], sc_bc[:, :]).then_inc(s_v2, 1)

            @block.scalar
            def _(scalar):
                # stat operand loads (HWDGE from the ACT sequencer)
                for j in range(NG):
                    sl = j % 2
                    if j >= 2:
                        # slot free once group j-2's wq product consumed it
                        scalar.wait_ge(s_vs, CF_J(j - 2) + 1)
                    scalar.dma_start(
                        tabg[sl][:, :], tabs[:, j * GW:(j + 1) * GW],
                    ).then_inc(s_tg, 16)
                    scalar.dma_start(
                        cntg[sl][:, :], cntr[:, j * GW:(j + 1) * GW],
                    ).then_inc(s_tg, 16)
                # sqrt(var + eps), full width
                scalar.wait_ge(s_vs, N_VAR)
                scalar.activation(
                    sh_bc[:, :], sh_bc[:, :],
                    mybir.ActivationFunctionType.Sqrt,
                    bias=eps_col[:, :],
                ).then_inc(s_ax, 1)
                # relu every tile
                for t in range(NT):
                    scalar.wait_ge(s_v2, t + 1)
                    gt = g2[t % NB2]
                    scalar.activation(
                        gt[:, FD:], num_sb[:, t * N_NUM:(t + 1) * N_NUM],
                        mybir.ActivationFunctionType.Copy,
                    )
                    scalar.activation(
                        gt[:, :FD], gt[:, :FD],
                        mybir.ActivationFunctionType.Relu,
                    ).then_inc(s_r2, 1)

            @block.tensor
            def _(tensor):
                for j in range(NG):
                    # per-partition (g, d) partials -> one 256-wide colsum
                    tensor.wait_ge(s_vs, W_J(j) + 4)
                    tensor.matmul(
                        ps_bc[0:1, j * 256:(j + 1) * 256], ones_c[:, :],
                        wg[:, :256], start=True, stop=True,
                    ).then_inc(s_pe, 1)
                    tensor.wait_ge(s_vs, W_J(j) + 9)
                    tensor.matmul(
                        ps_q[:, j * 256:(j + 1) * 256], ones_c[:, :],
                        wqb[:, :256], start=True, stop=True,
                    ).then_inc(s_pe, 1)
                # broadcast the moment rows across partitions (ps_bc reused;
                # the second broadcast waits for the mean copy to drain it)
                tensor.wait_ge(s_vs, N_LOOP + 2)
                for k in range(4):
                    mm = tensor.matmul(
                        ps_bc[:, k * 512:(k + 1) * 512], ones_r[:, :],
                        stat_s[:, k * 512:(k + 1) * 512], start=True, stop=True)
                mm.then_inc(s_pe, 1)
                tensor.wait_ge(s_vs, N_LOOP + 3)
                for k in range(4):
                    mm = tensor.matmul(
                        ps_bc[:, k * 512:(k + 1) * 512], ones_r[:, :],
                        stat_q[:, k * 512:(k + 1) * 512], start=True, stop=True)
                mm.then_inc(s_pe, 1)

        nc.compile()
    return nc


_NC_CACHE: list = []

# Optional profiling knobs (used by test harnesses; harmless defaults).
TRACE = False
TMPDIR = None
LAST_RESULT: list = []


def _get_nc():
    if not _NC_CACHE:
        _NC_CACHE.append(_build_nc())
    return _NC_CACHE[0]


def _host_prep_idx(cat_idx: np.ndarray) -> list[np.ndarray]:
    lin = cat_idx.astype(np.int64) + (np.arange(F, dtype=np.int64) * V)[None, :]
    lin = lin.astype(np.int16)                  # [B, F], values < 32032
    per_core = []
    for c in range(NCORES):
        sh = lin[c * BC:(c + 1) * BC].reshape(NT, TILE, F)
        # dma_gather order: gathered row i -> dst[i%128, i//128], so the
        # flat list per tile is [F, 128]; wrap by 16 and replicate to 128
        # partitions (8 gpsimd cores x 16).
        flat = sh.transpose(0, 2, 1).reshape(NT, NIDX)          # [NT, 4096]
        wrap = flat.reshape(NT, IW, 16).transpose(0, 2, 1)      # [NT, 16, IW]
        rep = np.broadcast_to(wrap[:, None, :, :], (NT, 8, 16, IW))
        per_core.append(np.ascontiguousarray(
            rep.reshape(NT, P, IW).transpose(1, 0, 2).reshape(P, NT * IW)))
    return per_core


def _host_prep_num(numerical: np.ndarray) -> list[np.ndarray]:
    out = []
    for c in range(NCORES):
        sh = numerical[c * BC:(c + 1) * BC].reshape(NT, TILE, N_NUM)
        out.append(np.ascontiguousarray(
            sh.transpose(1, 0, 2).reshape(P, NT * N_NUM)))
    return out


def _host_prep_stats(cat_idx: np.ndarray, tables: np.ndarray):
    """[s, g, d]-tiled padded table copy + d-replicated index histogram."""
    counts = np.zeros((F, VP), dtype=np.float32)
    for f in range(F):
        counts[f, :V] = np.bincount(cat_idx[:, f], minlength=V)
    tp = np.zeros((F, VP, D), dtype=np.float32)
    tp[:, :V, :] = tables
    # [j, g, s, p, d] -> [p, j, s, g, d]
    t5 = tp.reshape(NG, GF, 8, P, D).transpose(3, 0, 2, 1, 4)
    tabs = np.ascontiguousarray(t5.reshape(P, NG * GW))
    c4 = counts.reshape(NG, GF, 8, P).transpose(3, 0, 2, 1)     # [p, j, s, g]
    cntr = np.ascontiguousarray(
        np.repeat(c4[..., None], D, axis=4).reshape(P, NG * GW))
    return tabs, cntr


def kernel(cat_idx, numerical, tables, gamma, beta):
    cat_idx = np.asarray(cat_idx)
    numerical = np.asarray(numerical, dtype=np.float32)
    tables = np.asarray(tables, dtype=np.float32)
    gamma = np.asarray(gamma, dtype=np.float32)
    beta = np.asarray(beta, dtype=np.float32)

    nc = _get_nc()
    tab_flat = np.ascontiguousarray(tables.reshape(R, D))
    gb = np.ascontiguousarray(np.tile(np.concatenate(
        [gamma.reshape(FD), beta.reshape(FD)])[None, :], (P, 1)))
    idx_pc = _host_prep_idx(cat_idx)
    num_pc = _host_prep_num(numerical)
    tabs, cntr = _host_prep_stats(cat_idx, tables)

    in_maps = [
        {"tab": tab_flat, "idx": idx_pc[c], "nume": num_pc[c], "gb": gb,
         "tabs": tabs, "cntr": cntr}
        for c in range(NCORES)
    ]
    res = run_bass_kernel_spmd(nc, in_maps, core_ids=list(range(NCORES)),
                               trace=TRACE, tmpdir=TMPDIR)
    LAST_RESULT.clear()
    LAST_RESULT.append(res)
    out = np.concatenate([res.results[c]["out"] for c in range(NCORES)], axis=0)
    return out
